# revision 2
# baseline (speedup 1.0000x reference)
"""CenterLoss kernel for Trainium2 (8 NeuronCores, Bass/Tile).

Math: the reference builds the full [B, C] distance matrix, masks out every
column except labels[b] per row, clamps to [1e-12, 1e12] and sums. The masked
entries are exactly 0 before the clamp, so they each contribute 1e-12:

    loss = ( sum_b clip(||x_b - centers[labels_b]||^2, 1e-12, 1e12)
             + B*(C-1)*1e-12 ) / B

Device strategy: shard the batch over the 8 cores (256 rows each). Each core
holds the full `centers` in HBM, gathers its 256 label rows with an indirect
DMA (two 128-row gathers), computes row-wise squared distances, clamps,
reduces (free-dim reduce + ones-matmul partition reduce) to one scalar
partial. The host sums the 8 partials and adds the analytic clamp constant.
"""

import numpy as np

B, C, D = 2048, 100000, 64
N_CORES = 8
BS = B // N_CORES  # rows per core
J = BS // 128  # 128-row gather groups per core
CLAMP_MIN, CLAMP_MAX = 1e-12, 1e12

_cache: dict = {}


def _build():
    import concourse.bacc as bacc
    import concourse.bass as bass
    import concourse.mybir as mybir
    import concourse.tile as tile

    f32 = mybir.dt.float32
    i32 = mybir.dt.int32

    nc = bacc.Bacc(
        "TRN2", target_bir_lowering=False, debug=False, num_devices=N_CORES
    )

    xs = nc.dram_tensor("xs", [BS, D], f32, kind="ExternalInput")
    lbl = nc.dram_tensor("lbl", [128, J], i32, kind="ExternalInput")
    cen = nc.dram_tensor("centers", [C, D], f32, kind="ExternalInput")
    out = nc.dram_tensor("partial", [1, 1], f32, kind="ExternalOutput")

    with tile.TileContext(nc) as tc:
        with (
            tc.tile_pool(name="sb", bufs=2) as pool,
            tc.tile_pool(name="ps", bufs=1, space="PSUM") as psum,
        ):
            lbl_t = pool.tile([128, J], i32)
            nc.sync.dma_start(out=lbl_t[:], in_=lbl[:])
            ones = pool.tile([128, 1], f32)
            nc.vector.memset(ones[:], 1.0)
            acc = pool.tile([128, J], f32)
            for j in range(J):
                xt = pool.tile([128, D], f32, tag="xt")
                nc.sync.dma_start(out=xt[:], in_=xs[j * 128 : (j + 1) * 128, :])
                ct = pool.tile([128, D], f32, tag="ct")
                nc.gpsimd.indirect_dma_start(
                    out=ct[:],
                    out_offset=None,
                    in_=cen[:],
                    in_offset=bass.IndirectOffsetOnAxis(ap=lbl_t[:, j : j + 1], axis=0),
                )
                diff = pool.tile([128, D], f32, tag="diff")
                nc.vector.tensor_tensor(
                    out=diff[:], in0=xt[:], in1=ct[:], op=mybir.AluOpType.subtract
                )
                sq = pool.tile([128, D], f32, tag="sq")
                nc.vector.tensor_tensor(
                    out=sq[:], in0=diff[:], in1=diff[:], op=mybir.AluOpType.mult
                )
                nc.vector.tensor_reduce(
                    out=acc[:, j : j + 1],
                    in_=sq[:],
                    axis=mybir.AxisListType.X,
                    op=mybir.AluOpType.add,
                )
            accc = pool.tile([128, J], f32)
            nc.vector.tensor_scalar(
                out=accc[:],
                in0=acc[:],
                scalar1=float(CLAMP_MIN),
                scalar2=float(CLAMP_MAX),
                op0=mybir.AluOpType.max,
                op1=mybir.AluOpType.min,
            )
            rs = pool.tile([128, 1], f32)
            nc.vector.tensor_reduce(
                out=rs[:], in_=accc[:], axis=mybir.AxisListType.X, op=mybir.AluOpType.add
            )
            pt = psum.tile([1, 1], f32, space="PSUM")
            nc.tensor.matmul(out=pt[:], lhsT=ones[:], rhs=rs[:], start=True, stop=True)
            ot = pool.tile([1, 1], f32)
            nc.vector.tensor_copy(out=ot[:], in_=pt[:])
            nc.sync.dma_start(out=out[:], in_=ot[:])
    nc.compile()
    return nc


def _in_maps(x, centers, labels):
    x = np.ascontiguousarray(np.asarray(x), dtype=np.float32)
    centers = np.ascontiguousarray(np.asarray(centers), dtype=np.float32)
    lab = np.asarray(labels).astype(np.int64, copy=False)
    maps = []
    for k in range(N_CORES):
        sl = slice(k * BS, (k + 1) * BS)
        lbl_k = lab[sl].reshape(J, 128).T.astype(np.int32)
        maps.append(
            {
                "xs": x[sl],
                "lbl": np.ascontiguousarray(lbl_k),
                "centers": centers,
            }
        )
    return maps


def kernel(x, centers, labels, _return_results=False, _trace=False):
    from concourse.bass_utils import run_bass_kernel_spmd

    nc = _cache.get("nc")
    if nc is None:
        nc = _build()
        _cache["nc"] = nc

    res = run_bass_kernel_spmd(
        nc, _in_maps(x, centers, labels), list(range(N_CORES)), trace=_trace
    )
    total = sum(float(r["partial"][0, 0]) for r in res.results)
    total += B * (C - 1) * CLAMP_MIN
    loss = np.asarray(np.float32(total / B))
    if _return_results:
        return loss, res
    return loss
